# revision 1
# baseline (speedup 1.0000x reference)
"""Multi-head attention (B=2, S=2048, D=1024, H=16) on 8 trn2 NeuronCores.

Sharding: core c handles batch b = c//4 and heads 4*(c%4) .. 4*(c%4)+4
(tensor-parallel over heads, data-parallel over batch). Each core computes
its 4 heads' contribution to the output projection; the host sums the 4
partials per batch element and adds wo_b.

Layout strategy (all "T" tensors have the contraction dim on partitions):
  - host pre-transposes q,k,v -> qT/kT/vT (D, S) and mask -> binary bf16
    maskT (Sk, Sq), so the device never transposes anything.
  - projections produce qT_pair/kT_pair (128 = 2 heads x 64, Sq) and
    vp (Sk, 4 heads x [64 v-cols + ones-col]).
  - scores are computed transposed: alphaT (Sk-chunk, Sq-chunk) via two
    row-tiled K=64 matmuls (tile_position packs 2 heads onto the PE array).
  - softmax without max-subtraction (values are small): exp on ScalarE
    (PSUM -> SBUF bf16), binary-mask multiply on VectorE, and the softmax
    denominator comes free as row 64 of the PV matmul (ones column in vp).
  - PV: xT(65, Sq-chunk) accumulated over 16 Sk chunks in PSUM.
  - normalize with reciprocal + gpsimd partition_broadcast, then the output
    projection contracts 2 heads per matmul, accumulating both pairs.
"""

import numpy as np

B, S, D, H = 2, 2048, 1024, 16
DH = D // H  # 64
HEADS_PER_CORE = 4
N_CORES = 8
NQ = 4  # Sq chunks of 512
NSK = 16  # Sk chunks of 128
KC = 8  # D chunks of 128

_NC = None  # cached compiled bass program


def _build():
    import concourse.mybir as mybir
    import concourse.tile as tile
    from concourse import bacc

    F32 = mybir.dt.float32
    BF16 = mybir.dt.bfloat16
    P = 128

    nc = bacc.Bacc("TRN2")

    qT = nc.dram_tensor("qT", [D, S], F32, kind="ExternalInput")
    kT = nc.dram_tensor("kT", [D, S], F32, kind="ExternalInput")
    vT = nc.dram_tensor("vT", [D, S], F32, kind="ExternalInput")
    maskT = nc.dram_tensor("maskT", [S, S], BF16, kind="ExternalInput")
    wqT = nc.dram_tensor("wqT", [D, 256], F32, kind="ExternalInput")
    wkT = nc.dram_tensor("wkT", [D, 256], F32, kind="ExternalInput")
    wvT = nc.dram_tensor("wvT", [D, 256], F32, kind="ExternalInput")
    woT = nc.dram_tensor("woT", [256, D], F32, kind="ExternalInput")
    wqb = nc.dram_tensor("wqb", [256], F32, kind="ExternalInput")
    wkb = nc.dram_tensor("wkb", [256], F32, kind="ExternalInput")
    wvb = nc.dram_tensor("wvb", [256], F32, kind="ExternalInput")
    out = nc.dram_tensor("out", [S, D], F32, kind="ExternalOutput")

    AF = mybir.ActivationFunctionType
    MUL = mybir.AluOpType.mult
    ADD = mybir.AluOpType.add

    with tile.TileContext(nc) as tc:
        with (
            tc.tile_pool(name="persist", bufs=1) as persist,
            tc.tile_pool(name="vstream", bufs=10) as vstream,
            tc.tile_pool(name="qstream", bufs=4) as qstream,
            tc.tile_pool(name="pbuf", bufs=4) as pbuf,
            tc.tile_pool(name="obuf", bufs=3) as obuf,
            tc.tile_pool(name="nbuf", bufs=2) as nbuf,
        ):
            # ---- weights + biases (wvT chunk DMAs interleaved with the first
            # v-slice group below so the kc=0 matmul's deps land first) ----
            wvT_sb = persist.tile([P, KC, 256], F32, tag="wvT")
            wvb_sb = persist.tile([P, 256], F32, tag="wvb")

            with (
                tc.tile_pool(name="ps_proj", bufs=2, space="PSUM") as ps_proj,
                tc.tile_pool(name="ps_alpha", bufs=2, space="PSUM") as ps_alpha,
                tc.tile_pool(name="ps_xp", bufs=2, space="PSUM") as ps_xp,
            ):
                # ---- v projection: vp[sk] (128 Sk, 4 heads x 64) bf16 ----
                vp_sb = []
                for sk in range(NSK):
                    vp = persist.tile([P, 4, 65], BF16, tag=f"vp{sk}", name=f"vp{sk}")
                    nc.gpsimd.memset(vp[:], 1.0)  # ones column (col 64 per head)
                    vp_sb.append(vp)
                for sk2 in range(NSK // 2):
                    vsls = [
                        vstream.tile([P, 256], F32, tag="vsl", name=f"vsl{sk2}_{kc}")
                        for kc in range(KC)
                    ]
                    for kc in range(KC):
                        if sk2 == 0:
                            nc.sync.dma_start(
                                wvT_sb[:, kc], wvT[P * kc : P * (kc + 1), :]
                            )
                        nc.sync.dma_start(
                            vsls[kc][:],
                            vT[P * kc : P * (kc + 1), 256 * sk2 : 256 * (sk2 + 1)],
                        )
                    if sk2 == 0:
                        nc.sync.dma_start(
                            wvb_sb[:], wvb[:][None, :].to_broadcast((P, 256))
                        )
                    for skl in range(2):
                        sk = 2 * sk2 + skl
                        vp_ps = ps_proj.tile(
                            [P, 512], F32, tag="psproj", name=f"vpps{sk}"
                        )
                        for kc in range(KC):
                            nc.tensor.matmul(
                                vp_ps[:, 0:256],
                                vsls[kc][:, P * skl : P * (skl + 1)],
                                wvT_sb[:, kc],
                                start=(kc == 0),
                                stop=(kc == KC - 1),
                            )
                        # add bias and cast to bf16
                        nc.vector.tensor_tensor(
                            vp_sb[sk][:, :, 0:64],
                            vp_ps[:, 0:256].rearrange("p (h d) -> p h d", h=4),
                            wvb_sb[:].rearrange("p (h d) -> p h d", h=4),
                            ADD,
                        )

                # ---- remaining weights (needed after vproj) ----
                wqT_sb = persist.tile([P, KC, 256], F32, tag="wqT")
                nc.sync.dma_start(
                    wqT_sb[:], wqT[:].rearrange("(kc p) m -> p kc m", p=P)
                )
                wkT_sb = persist.tile([P, KC, 256], F32, tag="wkT")
                nc.sync.dma_start(
                    wkT_sb[:], wkT[:].rearrange("(kc p) m -> p kc m", p=P)
                )
                woT_sb = persist.tile([P, 2, D], F32, tag="woT")
                nc.sync.dma_start(
                    woT_sb[:], woT[:].rearrange("(pr p) m -> p pr m", p=P)
                )
                wqb_sb = persist.tile([P, 2], F32, tag="wqb")
                nc.sync.dma_start(wqb_sb[:], wqb[:].rearrange("(pr p) -> p pr", p=P))
                wkb_sb = persist.tile([P, 2], F32, tag="wkb")
                nc.sync.dma_start(wkb_sb[:], wkb[:].rearrange("(pr p) -> p pr", p=P))

                # ---- mask tiles (binary bf16, [Sk-part, Sq]); tile 0 DMA'd
                # ahead of the q/k stream so attention never waits on it ----
                mask_sb = [
                    persist.tile([P, S], BF16, tag=f"mask{sk}", name=f"mask{sk}")
                    for sk in range(NSK)
                ]
                nc.sync.dma_start(mask_sb[0][:], maskT[0:P, :])

                # ---- q/k projections -> per-chunk qT/kT pair tiles ----
                qTp = [
                    [
                        persist.tile(
                            [P, 512], F32, tag=f"qTp{p}_{nq}", name=f"qTp{p}_{nq}"
                        )
                        for nq in range(NQ)
                    ]
                    for p in range(2)
                ]
                kTp = [
                    [
                        persist.tile(
                            [P, 512], F32, tag=f"kTp{p}_{nq}", name=f"kTp{p}_{nq}"
                        )
                        for nq in range(NQ)
                    ]
                    for p in range(2)
                ]
                for src, wsb, bsb, dst, nm in (
                    (qT, wqT_sb, wqb_sb, qTp, "q"),
                    (kT, wkT_sb, wkb_sb, kTp, "k"),
                ):
                    for nq in range(NQ):
                        pps = [
                            ps_proj.tile(
                                [P, 512], F32, tag="psproj", name=f"{nm}ps{nq}_{p}"
                            )
                            for p in range(2)
                        ]
                        for kc in range(KC):
                            xsl = qstream.tile(
                                [P, 512], F32, tag="xsl", name=f"{nm}sl{nq}_{kc}"
                            )
                            nc.sync.dma_start(
                                xsl[:],
                                src[P * kc : P * (kc + 1), 512 * nq : 512 * (nq + 1)],
                            )
                            for p in range(2):
                                nc.tensor.matmul(
                                    pps[p][:],
                                    wsb[:, kc, 128 * p : 128 * (p + 1)],
                                    xsl[:],
                                    start=(kc == 0),
                                    stop=(kc == KC - 1),
                                )
                        for p in range(2):
                            # psum -> sbuf with per-partition bias add
                            nc.scalar.activation(
                                dst[p][nq][:],
                                pps[p][:],
                                AF.Identity,
                                bias=bsb[:, p : p + 1],
                            )

                # ---- remaining mask tiles (tile 0 prefetched above) ----
                for sk in range(1, NSK):
                    nc.sync.dma_start(
                        mask_sb[sk][:], maskT[P * sk : P * (sk + 1), :]
                    )

                # ---- attention + normalization + output projection ----
                xnorm = [
                    [
                        persist.tile(
                            [P, 512], F32, tag=f"xn{p}_{nq}", name=f"xn{p}_{nq}"
                        )
                        for nq in range(NQ)
                    ]
                    for p in range(2)
                ]
                for nq in range(NQ):
                    for pr in range(2):
                        xps = [
                            ps_xp.tile(
                                [P, 512], F32, tag="xps", name=f"xps{nq}_{pr}_{h}"
                            )
                            for h in range(2)
                        ]
                        for sk in range(NSK):
                            alpha = ps_alpha.tile(
                                [P, 1024], F32, tag="alpha", name=f"al{nq}_{pr}_{sk}"
                            )
                            for h in range(2):
                                nc.tensor.matmul(
                                    alpha[:, 512 * h : 512 * (h + 1)],
                                    kTp[pr][sk // 4][
                                        64 * h : 64 * h + 64,
                                        P * (sk % 4) : P * (sk % 4 + 1),
                                    ],
                                    qTp[pr][nq][64 * h : 64 * h + 64, :],
                                    start=True,
                                    stop=True,
                                    tile_position=(64 * h, 0),
                                )
                            psb = pbuf.tile(
                                [P, 1024],
                                BF16,
                                tag="psb",
                                name=f"psb{nq}_{pr}_{sk}",
                            )
                            nc.scalar.activation(psb[:], alpha[:], AF.Exp)
                            nc.vector.tensor_tensor(
                                psb[:].rearrange("p (h n) -> p h n", h=2),
                                psb[:].rearrange("p (h n) -> p h n", h=2),
                                mask_sb[sk][:, 512 * nq : 512 * (nq + 1)][
                                    :, None, :
                                ].to_broadcast((P, 2, 512)),
                                MUL,
                            )
                            for h in range(2):
                                nc.tensor.matmul(
                                    xps[h][0:65, :],
                                    vp_sb[sk][:, 2 * pr + h],
                                    psb[:, 512 * h : 512 * (h + 1)],
                                    start=(sk == 0),
                                    stop=(sk == NSK - 1),
                                )
                        rs, rbs = [], []
                        for h in range(2):
                            r = nbuf.tile(
                                [1, 512], F32, tag=f"r{h}", name=f"r{nq}_{pr}_{h}"
                            )
                            nc.vector.reciprocal(r[:], xps[h][64:65, :])
                            rs.append(r)
                        for h in range(2):
                            rb = nbuf.tile(
                                [64, 512], F32, tag=f"rb{h}", name=f"rb{nq}_{pr}_{h}"
                            )
                            nc.gpsimd.partition_broadcast(rb[:], rs[h][:])
                            rbs.append(rb)
                        for h in range(2):
                            nc.vector.tensor_tensor(
                                xnorm[pr][nq][64 * h : 64 * h + 64, :],
                                xps[h][0:64, :],
                                rbs[h][:],
                                MUL,
                            )

                    # ---- output projection for this Sq chunk (fills PE slack
                    # while the next chunk's attention is ACT-bound) ----
                    for ml in range(4):
                        m = 4 * nq + ml
                        osb = obuf.tile([P, D], F32, tag="osb", name=f"osb{m}")
                        for d in range(2):
                            ops = ps_proj.tile(
                                [P, 512], F32, tag="psproj", name=f"ops{m}_{d}"
                            )
                            for pr2 in range(2):
                                nc.tensor.matmul(
                                    ops[:],
                                    xnorm[pr2][nq][:, P * ml : P * (ml + 1)],
                                    woT_sb[:, pr2, 512 * d : 512 * (d + 1)],
                                    start=(pr2 == 0),
                                    stop=(pr2 == 1),
                                )
                            nc.any.tensor_copy(
                                out=osb[:, 512 * d : 512 * (d + 1)], in_=ops[:]
                            )
                        nc.sync.dma_start(out[P * m : P * (m + 1), :], osb[:])

    nc.finalize()
    return nc


def _get_nc():
    global _NC
    if _NC is None:
        _NC = _build()
    return _NC


def _prep_inputs(q, k, v, mask, wq_w, wq_b, wk_w, wk_b, wv_w, wv_b, wo_w, wo_b):
    import ml_dtypes

    f32 = np.float32
    q = np.asarray(q, f32)
    k = np.asarray(k, f32)
    v = np.asarray(v, f32)
    mask = np.asarray(mask)
    wq_w = np.asarray(wq_w, f32)
    wk_w = np.asarray(wk_w, f32)
    wv_w = np.asarray(wv_w, f32)
    wo_w = np.asarray(wo_w, f32)

    qTb = [np.ascontiguousarray(q[b].T) for b in range(B)]
    kTb = [np.ascontiguousarray(k[b].T) for b in range(B)]
    vTb = [np.ascontiguousarray(v[b].T) for b in range(B)]
    maskTb = [
        np.ascontiguousarray((~mask[b, 0]).T).astype(ml_dtypes.bfloat16)
        for b in range(B)
    ]

    in_maps = []
    for c in range(N_CORES):
        b = c // 4
        g = c % 4
        rows = slice(256 * g, 256 * (g + 1))
        in_maps.append(
            {
                "qT": qTb[b],
                "kT": kTb[b],
                "vT": vTb[b],
                "maskT": maskTb[b],
                "wqT": np.ascontiguousarray(wq_w[rows, :].T),
                "wkT": np.ascontiguousarray(wk_w[rows, :].T),
                "wvT": np.ascontiguousarray(wv_w[rows, :].T),
                "woT": np.ascontiguousarray(wo_w[:, rows].T),
                "wqb": np.ascontiguousarray(np.asarray(wq_b, f32)[rows]),
                "wkb": np.ascontiguousarray(np.asarray(wk_b, f32)[rows]),
                "wvb": np.ascontiguousarray(np.asarray(wv_b, f32)[rows]),
            }
        )
    return in_maps


def run(inputs, trace=False):
    """Run the kernel; returns (output, BassKernelResults)."""
    from concourse.bass_utils import run_bass_kernel_spmd

    in_maps = _prep_inputs(**inputs)
    nc = _get_nc()
    res = None
    last_exc = None
    for attempt in range(3):
        try:
            res = run_bass_kernel_spmd(
                nc, in_maps, core_ids=list(range(N_CORES)), trace=trace
            )
            break
        except Exception as e:  # transient device/tunnel failures
            last_exc = e
            try:
                import jax

                jax.clear_caches()
                try:
                    jax.extend.backend.clear_backends()
                except Exception:
                    from jax._src import api as _jax_api

                    _jax_api.clear_backends()
            except Exception:
                pass
            import time as _time

            _time.sleep(2.0 * (attempt + 1))
    if res is None:
        raise last_exc
    wo_b = np.asarray(inputs["wo_b"], np.float32)
    out = np.zeros((B, S, D), np.float32)
    for b in range(B):
        acc = np.zeros((S, D), np.float32)
        for g in range(4):
            acc += res.results[4 * b + g]["out"]
        out[b] = acc + wo_b[None, :]
    return out, res


def kernel(**inputs) -> np.ndarray:
    out, _ = run(inputs, trace=False)
    return out



# revision 26
# speedup vs baseline: 2.5967x; 2.5967x over previous
"""Multi-head attention (B=2, S=2048, D=1024, H=16) on 8 trn2 NeuronCores.

Sharding: core c handles batch b = c//4 and heads 4*(c%4) .. 4*(c%4)+4
(tensor-parallel over heads, data-parallel over batch). Each core computes
its 4 heads' contribution to the output projection; the host sums the 4
partials per batch element and adds wo_b.

All matmuls run in bf16 (inputs/weights cast on host, intermediates cast
on-chip); PSUM accumulation stays f32. Engine placement: PE does all
matmuls; ACT does only the softmax exp (the pacer, ~1.04us/slot); DVE
does the mask-multiply, q/k bias adds, reciprocal and normalize
multiplies; Pool (gpsimd) does the v-bias add, partition broadcasts and
PSUM->SBUF output copies.

Flat-slot software pipeline: slot s emits QK(s)+exp+mask-mul, PV(s-lag)
(lag tapers 10->4 to ride out late mask/v DMAs), one filler matmul
(vproj early, then qproj of later Sq chunks / oproj of finished chunks).
The cost model collapses the PE clock permanently after a single >~4us
idle gap, so the PE queue is kept stocked; gaps under ~3.3us are free.
The mask is DMA'd as 64 [128,512] column tiles so pass-0 mask arrival
(~0.63us/tile) outruns consumption (~1.04us/tile); k/q/v projection PSUM
tiles share the alpha pool's [P,1024] buffers to avoid ping-pong stalls.
"""

import numpy as np

B, S, D, H = 2, 2048, 1024, 16
DH = D // H  # 64
HEADS_PER_CORE = 4
N_CORES = 8
NQ = 4  # Sq chunks of 512
NSK = 16  # Sk chunks of 128
KC = 8  # D chunks of 128

_NC = None  # cached compiled bass program


def _build():
    import concourse.mybir as mybir
    import concourse.tile as tile
    from concourse import bacc

    F32 = mybir.dt.float32
    BF16 = mybir.dt.bfloat16
    P = 128

    nc = bacc.Bacc("TRN2")

    qT = nc.dram_tensor("qT", [D, S], BF16, kind="ExternalInput")
    kT = nc.dram_tensor("kT", [D, S], BF16, kind="ExternalInput")
    vT = nc.dram_tensor("vT", [D, S], BF16, kind="ExternalInput")
    maskT = nc.dram_tensor("maskT", [S, S], BF16, kind="ExternalInput")
    wqT = nc.dram_tensor("wqT", [D, 256], BF16, kind="ExternalInput")
    wkT = nc.dram_tensor("wkT", [D, 256], BF16, kind="ExternalInput")
    wvT = nc.dram_tensor("wvT", [D, 256], BF16, kind="ExternalInput")
    woT = nc.dram_tensor("woT", [256, D], BF16, kind="ExternalInput")
    wqb = nc.dram_tensor("wqb", [256], F32, kind="ExternalInput")
    wkb = nc.dram_tensor("wkb", [256], F32, kind="ExternalInput")
    wvb = nc.dram_tensor("wvb", [256], F32, kind="ExternalInput")
    out = nc.dram_tensor("out", [S, D], BF16, kind="ExternalOutput")

    AF = mybir.ActivationFunctionType
    MUL = mybir.AluOpType.mult
    ADD = mybir.AluOpType.add

    with tile.TileContext(nc) as tc:
        with (
            tc.tile_pool(name="persist", bufs=1) as persist,
            tc.tile_pool(name="kst", bufs=8) as kst,
            tc.tile_pool(name="vst", bufs=16) as vst,
            tc.tile_pool(name="qs", bufs=16) as qs,
            tc.tile_pool(name="mcp", bufs=32) as mcp,
            tc.tile_pool(name="pbuf", bufs=14) as pbuf,
            tc.tile_pool(name="obuf", bufs=2) as obuf,
            tc.tile_pool(name="nbuf", bufs=1) as nbuf,
            tc.tile_pool(name="ps_proj", bufs=2, space="PSUM") as ps_proj,
            tc.tile_pool(name="ps_alpha", bufs=2, space="PSUM") as ps_alpha,
            tc.tile_pool(name="ps_xp", bufs=2, space="PSUM") as ps_xp,
        ):
            # ---------------- DMA stream 1: k, q(nq0), v halves -------------
            wkT_sb = persist.tile([P, KC, 256], BF16, tag="wkT")
            nc.sync.dma_start(wkT_sb[:], wkT[:].rearrange("(kc p) m -> p kc m", p=P))
            wkb_sb = persist.tile([P, 2], F32, tag="wkb")
            nc.sync.dma_start(wkb_sb[:], wkb[:].rearrange("(pr p) -> p pr", p=P))
            kch = []
            for kc in range(KC):
                t = kst.tile([P, S], BF16, tag="kch", name=f"kch{kc}")
                nc.sync.dma_start(t[:], kT[P * kc : P * (kc + 1), :])
                kch.append(t)

            wqT_sb = persist.tile([P, KC, 256], BF16, tag="wqT")
            nc.sync.dma_start(wqT_sb[:], wqT[:].rearrange("(kc p) m -> p kc m", p=P))
            wqb_sb = persist.tile([P, 2], F32, tag="wqb")
            nc.sync.dma_start(wqb_sb[:], wqb[:].rearrange("(pr p) -> p pr", p=P))
            qch0 = []
            for kc in range(KC):
                t = qs.tile([P, 512], BF16, tag="qs", name=f"q0_{kc}")
                nc.sync.dma_start(t[:], qT[P * kc : P * (kc + 1), 0:512])
                qch0.append(t)

            wvT_sb = persist.tile([P, KC, 256], BF16, tag="wvT")
            nc.sync.dma_start(wvT_sb[:], wvT[:].rearrange("(kc p) m -> p kc m", p=P))
            wvb_sb = persist.tile([P, 256], F32, tag="wvb")
            nc.sync.dma_start(wvb_sb[:], wvb[:][None, :].to_broadcast((P, 256)))

            # mask column tiles: mask_c[c][sk] covers maskT rows of Sk chunk
            # sk, Sq columns [512c, 512c+512)
            mask_c = [[None] * NSK for _ in range(NQ)]

            def dma_mask_col(c, lo=0, hi=NSK):
                for sk in range(lo, hi):
                    t = mcp.tile([P, 512], BF16, tag="mc", name=f"mc{c}_{sk}")
                    nc.sync.dma_start(
                        t[:], maskT[P * sk : P * (sk + 1), 512 * c : 512 * (c + 1)]
                    )
                    mask_c[c][sk] = t

            vch = [[None] * KC for _ in range(2)]

            def dma_v_half(half):
                for kc in range(KC):
                    t = vst.tile([P, 1024], BF16, tag="vch", name=f"vch{half}_{kc}")
                    nc.sync.dma_start(
                        t[:], vT[P * kc : P * (kc + 1), 1024 * half : 1024 * (half + 1)]
                    )
                    vch[half][kc] = t

            # interleave mask col 0 with the v halves: both gate the PVs
            dma_mask_col(0, 0, 8)
            dma_v_half(0)
            dma_mask_col(0, 8, NSK)
            dma_v_half(1)

            # ---------------- k projection (kps in the alpha pool) ----------
            # Two kc-outer passes (p0 then p1) so pass A overlaps the k DMA
            # arrivals; each pass holds two [P,1024] alpha-pool tiles, each
            # packing two nq chunks' 512-wide halves.
            kTp = [
                [
                    persist.tile([P, 512], BF16, tag=f"kTp{p}_{nq}", name=f"kTp{p}_{nq}")
                    for nq in range(NQ)
                ]
                for p in range(2)
            ]
            vp_sb = []
            for sk in range(NSK):
                vp = persist.tile([P, 4, 65], BF16, tag=f"vp{sk}", name=f"vp{sk}")
                nc.gpsimd.memset(vp[:], 1.0)  # ones column (col 64 per head)
                vp_sb.append(vp)
            # pass A (p0): two [P,1024] alpha-pool tiles, kc-outer so it is
            # paced by the k-chunk DMA arrivals. pass B (p1) interleaves one
            # kc behind pass A, filling the PE's DMA-pacing gaps; its PSUM
            # tiles live in ps_xp/ps_proj (both idle during projections) so
            # nothing ping-pongs on the alpha buffers.
            kpsA = [
                ps_alpha.tile([P, 1024], F32, tag="alpha", name=f"kpsA{pair}")
                for pair in range(2)
            ]
            kpsB = [
                ps_xp.tile([P, 512], F32, tag="xps", name="kpsB0")[:],
                ps_xp.tile([P, 512], F32, tag="xps", name="kpsB1")[:],
                ps_proj.tile([P, 512], F32, tag="psproj", name="kpsB2")[:],
                ps_proj.tile([P, 512], F32, tag="psproj", name="kpsB3")[:],
            ]

            def kproj_mm(p, nq, kc, dst):
                nc.tensor.matmul(
                    dst,
                    wkT_sb[:, kc, 128 * p : 128 * (p + 1)],
                    kch[kc][:, 512 * nq : 512 * (nq + 1)],
                    start=(kc == 0),
                    stop=(kc == KC - 1),
                )

            for kc in range(KC):
                for nq in range(NQ):
                    kproj_mm(
                        0, nq, kc, kpsA[nq // 2][:, 512 * (nq % 2) : 512 * (nq % 2 + 1)]
                    )
                if kc > 0:
                    for nq in range(NQ):
                        kproj_mm(1, nq, kc - 1, kpsB[nq])
            for nq in range(NQ):
                kproj_mm(1, nq, KC - 1, kpsB[nq])
            for nq in range(NQ):
                nc.vector.tensor_scalar(
                    kTp[0][nq][:],
                    kpsA[nq // 2][:, 512 * (nq % 2) : 512 * (nq % 2 + 1)],
                    wkb_sb[:, 0:1],
                    None,
                    ADD,
                )
            for nq in range(NQ):
                nc.vector.tensor_scalar(
                    kTp[1][nq][:], kpsB[nq], wkb_sb[:, 1:2], None, ADD
                )

            # ---------------- q projection ----------------
            qTp = [
                [
                    persist.tile([P, 512], BF16, tag=f"qTp{p}_{nq}", name=f"qTp{p}_{nq}")
                    for nq in range(NQ)
                ]
                for p in range(2)
            ]

            def qproj_half_units(nq, qtiles, col_off, pps, p):
                """Per-kc units projecting one p-half of q chunk nq."""

                def mk(kc):
                    def emit():
                        nc.tensor.matmul(
                            pps,
                            wqT_sb[:, kc, 128 * p : 128 * (p + 1)],
                            qtiles[kc][:, col_off : col_off + 512],
                            start=(kc == 0),
                            stop=(kc == KC - 1),
                        )
                        if kc == KC - 1:
                            nc.vector.tensor_scalar(
                                qTp[p][nq][:],
                                pps,
                                wqb_sb[:, p : p + 1],
                                None,
                                ADD,
                            )

                    return emit

                return [mk(kc) for kc in range(KC)]

            def qproj_units(nq, qtiles, col_off):
                pps = [
                    ps_proj.tile([P, 512], F32, tag="psproj", name=f"qps{nq}_{p}")[:]
                    for p in range(2)
                ]
                u0 = qproj_half_units(nq, qtiles, col_off, pps[0], 0)
                u1 = qproj_half_units(nq, qtiles, col_off, pps[1], 1)
                return [
                    (lambda a=a, b=b: (a(), b(), None)[2]) for a, b in zip(u0, u1)
                ]

            # p0 half inline (QK block (0,0) only needs qTp[0][0]); p1 half
            # becomes the first filler units inside the attention stream.
            qps0 = [
                ps_proj.tile([P, 512], F32, tag="psproj", name=f"qps0_{p}")[:]
                for p in range(2)
            ]
            for u in qproj_half_units(0, qch0, 0, qps0[0], 0):
                u()
            q0p1_units = qproj_half_units(0, qch0, 0, qps0[1], 1)

            # stream-2 DMAs are emitted inside the slot loop (sched entries)
            # so buffer-recycling dependencies match emission order.
            woT_sb = persist.tile([P, 2, D], BF16, tag="woT")
            qch1, qch2, qch3 = [], [], []

            def dma_q_chunk(dst, col0):
                for kc in range(KC):
                    t = qs.tile([P, 512], BF16, tag="qs", name=f"q{col0}_{kc}")
                    nc.sync.dma_start(
                        t[:], qT[P * kc : P * (kc + 1), col0 : col0 + 512]
                    )
                    dst.append(t)

            # ------------- flat-slot attention pipeline + fillers -----------
            xnorm = [
                [
                    persist.tile([P, 512], BF16, tag=f"xn{p}_{nq}", name=f"xn{p}_{nq}")
                    for nq in range(NQ)
                ]
                for p in range(2)
            ]

            def vproj_unit(sk):
                half, skl = sk // 8, sk % 8

                def emit():
                    vp_ps = ps_proj.tile([P, 512], F32, tag="psproj", name=f"vpps{sk}")
                    for kc in range(KC):
                        nc.tensor.matmul(
                            vp_ps[:, 0:256],
                            vch[half][kc][:, P * skl : P * (skl + 1)],
                            wvT_sb[:, kc],
                            start=(kc == 0),
                            stop=(kc == KC - 1),
                        )
                    nc.vector.tensor_tensor(
                        vp_sb[sk][:, :, 0:64],
                        vp_ps[:, 0:256].rearrange("p (h d) -> p h d", h=4),
                        wvb_sb[:].rearrange("p (h d) -> p h d", h=4),
                        ADD,
                    )

                return emit

            def oproj_units(nq):
                units = []
                for ml in range(4):
                    m = 4 * nq + ml
                    osb = obuf.tile([P, D], BF16, tag="osb", name=f"osb{m}")
                    for d in range(2):
                        ops = ps_proj.tile(
                            [P, 512], F32, tag="psproj", name=f"ops{m}_{d}"
                        )

                        def emit_mm(pr2, m=m, ml=ml, d=d, osb=osb, ops=ops):
                            nc.tensor.matmul(
                                ops[:],
                                xnorm[pr2][nq][:, P * ml : P * (ml + 1)],
                                woT_sb[:, pr2, 512 * d : 512 * (d + 1)],
                                start=(pr2 == 0),
                                stop=(pr2 == 1),
                            )
                            if pr2 == 1:
                                dst = osb[:, 512 * d : 512 * (d + 1)]
                                if nq == NQ - 1:
                                    nc.scalar.copy(out=dst, in_=ops[:])
                                else:
                                    nc.vector.tensor_copy(out=dst, in_=ops[:])
                                if d == 1:
                                    nc.sync.dma_start(
                                        out[P * m : P * (m + 1), :], osb[:]
                                    )

                        units.append(lambda f=emit_mm: f(0))
                        units.append(lambda f=emit_mm: f(1))
                return units

            # filler schedule: (slot, lazy factory). DMA batches are also
            # sched units: emitting them inside the slot stream keeps the
            # buffer-recycle dependencies consistent with their readers.
            def dma_unit(fn):
                return lambda: [fn]

            sched = []
            sched.append((0, lambda: q0p1_units))
            sched.append((8, dma_unit(lambda: dma_q_chunk(qch1, 512))))
            for sk in range(NSK):
                sched.append((10 + sk, lambda sk=sk: [vproj_unit(sk)]))
            sched.append((10, dma_unit(lambda: dma_mask_col(1))))
            sched.append(
                (
                    11,
                    dma_unit(
                        lambda: (
                            nc.sync.dma_start(
                                woT_sb[:],
                                woT[:].rearrange("(pr p) m -> p pr m", p=P),
                            ),
                            dma_q_chunk(qch2, 1024),
                        )
                    ),
                )
            )
            sched.append((26, lambda: qproj_units(1, qch1, 0)))
            sched.append((32, dma_unit(lambda: dma_q_chunk(qch3, 1536))))
            sched.append((33, dma_unit(lambda: dma_mask_col(2))))
            sched.append((43, lambda: qproj_units(2, qch2, 0)))
            sched.append((52, lambda: oproj_units(0)))
            sched.append((64, dma_unit(lambda: dma_mask_col(3))))
            sched.append((69, lambda: qproj_units(3, qch3, 0)))
            sched.append((78, lambda: oproj_units(1)))
            sched.append((101, lambda: oproj_units(2)))
            sched.sort(key=lambda x: x[0])

            # PV slot map: big lag early (rides out late mask/v DMA), base 4
            # in steady state, +3 cushion at block starts so the previous
            # block's normalize chain can release the xps buffers in time.
            pv_slot = []
            for i in range(128):
                base = i + (12 if i <= 15 else 4)
                prev = pv_slot[-1] if pv_slot else 0
                if i % 16 == 0 and i > 0:
                    prev += 3
                pv_slot.append(max(base, prev))

            # slot -> list of PV indices
            pv_at = {}
            for i in range(128):
                pv_at.setdefault(pv_slot[i], []).append(i)

            psb_store = {}
            xps_store = {}
            pending = []
            si = 0

            def normalize(b):
                nq, pr = b // 2, b % 2
                xps = xps_store.pop(b)
                rs, rbs = [], []
                for h in range(2):
                    r = nbuf.tile([1, 512], F32, tag=f"r{h}", name=f"r{nq}_{pr}_{h}")
                    nc.vector.reciprocal(r[:], xps[h][64:65, :])
                    rs.append(r)
                for h in range(2):
                    rb = nbuf.tile(
                        [64, 512], F32, tag=f"rb{h}", name=f"rb{nq}_{pr}_{h}"
                    )
                    nc.gpsimd.partition_broadcast(rb[:], rs[h][:])
                    rbs.append(rb)
                for h in range(2):
                    nc.vector.tensor_tensor(
                        xnorm[pr][nq][64 * h : 64 * h + 64, :],
                        xps[h][0:64, :],
                        rbs[h][:],
                        MUL,
                    )

            NSLOT = pv_slot[-1] + 1
            for s in range(NSLOT):
                if s < 128:
                    b, sk = s // 16, s % 16
                    nq, pr = b // 2, b % 2
                    alpha = ps_alpha.tile(
                        [P, 1024], F32, tag="alpha", name=f"al{b}_{sk}"
                    )
                    for h in range(2):
                        nc.tensor.matmul(
                            alpha[:, 512 * h : 512 * (h + 1)],
                            kTp[pr][sk // 4][
                                64 * h : 64 * h + 64,
                                P * (sk % 4) : P * (sk % 4 + 1),
                            ],
                            qTp[pr][nq][64 * h : 64 * h + 64, :],
                            start=True,
                            stop=True,
                            tile_position=(64 * h, 0),
                        )
                    psb = pbuf.tile([P, 1024], BF16, tag="psb", name=f"psb{b}_{sk}")
                    nc.scalar.activation(psb[:], alpha[:], AF.Exp)
                    nc.vector.tensor_tensor(
                        psb[:].rearrange("p (h n) -> p h n", h=2),
                        psb[:].rearrange("p (h n) -> p h n", h=2),
                        mask_c[nq][sk][:][:, None, :].to_broadcast((P, 2, 512)),
                        MUL,
                    )
                    psb_store[s] = psb
                for i in pv_at.get(s, []):
                    b, sk = i // 16, i % 16
                    nq, pr = b // 2, b % 2
                    if sk == 0:
                        xps_store[b] = [
                            ps_xp.tile([P, 512], F32, tag="xps", name=f"xps{b}_{h}")
                            for h in range(2)
                        ]
                    psb = psb_store.pop(i)
                    for h in range(2):
                        nc.tensor.matmul(
                            xps_store[b][h][0:65, :],
                            vp_sb[sk][:, 2 * pr + h],
                            psb[:, 512 * h : 512 * (h + 1)],
                            start=(sk == 0),
                            stop=(sk == NSK - 1),
                        )
                    if sk == NSK - 1:
                        normalize(b)
                while si < len(sched) and sched[si][0] <= s:
                    pending.extend(sched[si][1]())
                    si += 1
                n_emit = min(2, len(pending))
                for _ in range(n_emit):
                    pending.pop(0)()
            while si < len(sched):
                pending.extend(sched[si][1]())
                si += 1
            for u in pending:
                u()
            for u in oproj_units(3):
                u()

    nc.finalize()
    return nc


def _get_nc():
    global _NC
    if _NC is None:
        _NC = _build()
    return _NC


def _prep_inputs(q, k, v, mask, wq_w, wq_b, wk_w, wk_b, wv_w, wv_b, wo_w, wo_b):
    import ml_dtypes

    f32 = np.float32
    bf16 = ml_dtypes.bfloat16
    q = np.asarray(q, f32)
    k = np.asarray(k, f32)
    v = np.asarray(v, f32)
    mask = np.asarray(mask)
    wq_w = np.asarray(wq_w, f32)
    wk_w = np.asarray(wk_w, f32)
    wv_w = np.asarray(wv_w, f32)
    wo_w = np.asarray(wo_w, f32)

    qTb = [np.ascontiguousarray(q[b].T).astype(bf16) for b in range(B)]
    kTb = [np.ascontiguousarray(k[b].T).astype(bf16) for b in range(B)]
    vTb = [np.ascontiguousarray(v[b].T).astype(bf16) for b in range(B)]
    maskTb = [
        np.ascontiguousarray((~mask[b, 0]).T).astype(bf16) for b in range(B)
    ]

    in_maps = []
    for c in range(N_CORES):
        b = c // 4
        g = c % 4
        rows = slice(256 * g, 256 * (g + 1))
        in_maps.append(
            {
                "qT": qTb[b],
                "kT": kTb[b],
                "vT": vTb[b],
                "maskT": maskTb[b],
                "wqT": np.ascontiguousarray(wq_w[rows, :].T).astype(bf16),
                "wkT": np.ascontiguousarray(wk_w[rows, :].T).astype(bf16),
                "wvT": np.ascontiguousarray(wv_w[rows, :].T).astype(bf16),
                "woT": np.ascontiguousarray(wo_w[:, rows].T).astype(bf16),
                "wqb": np.ascontiguousarray(np.asarray(wq_b, f32)[rows]),
                "wkb": np.ascontiguousarray(np.asarray(wk_b, f32)[rows]),
                "wvb": np.ascontiguousarray(np.asarray(wv_b, f32)[rows]),
            }
        )
    return in_maps


def run(inputs, trace=False):
    """Run the kernel; returns (output, BassKernelResults)."""
    from concourse.bass_utils import run_bass_kernel_spmd

    in_maps = _prep_inputs(**inputs)
    nc = _get_nc()
    res = None
    last_exc = None
    for attempt in range(3):
        try:
            res = run_bass_kernel_spmd(
                nc, in_maps, core_ids=list(range(N_CORES)), trace=trace
            )
            break
        except Exception as e:  # transient device/tunnel failures
            last_exc = e
            try:
                import jax

                jax.clear_caches()
                try:
                    jax.extend.backend.clear_backends()
                except Exception:
                    from jax._src import api as _jax_api

                    _jax_api.clear_backends()
            except Exception:
                pass
            import time as _time

            _time.sleep(2.0 * (attempt + 1))
    if res is None:
        raise last_exc
    wo_b = np.asarray(inputs["wo_b"], np.float32)
    out = np.zeros((B, S, D), np.float32)
    for b in range(B):
        acc = np.zeros((S, D), np.float32)
        for g in range(4):
            acc += np.asarray(res.results[4 * b + g]["out"], np.float32)
        out[b] = acc + wo_b[None, :]
    return out, res


def kernel(**inputs) -> np.ndarray:
    out, _ = run(inputs, trace=False)
    return out


# revision 34
# speedup vs baseline: 2.7121x; 1.0444x over previous
"""Multi-head attention (B=2, S=2048, D=1024, H=16) on 8 trn2 NeuronCores.

Sharding: core c handles batch b = c//4 and heads 4*(c%4) .. 4*(c%4)+4
(tensor-parallel over heads, data-parallel over batch). Each core computes
its 4 heads' contribution to the output projection; the host sums the 4
partials per batch element and adds wo_b.

All matmuls run in bf16 (inputs/weights cast on host, intermediates cast
on-chip); PSUM accumulation stays f32. Engine placement: PE does all
matmuls; ACT does only the softmax exp (the pacer, ~1.04us/slot); DVE
does the mask-multiply, q/k bias adds, reciprocal and normalize
multiplies; Pool (gpsimd) does the v-bias add, partition broadcasts and
PSUM->SBUF output copies.

Flat-slot software pipeline: slot s emits QK(s)+exp+mask-mul, PV(s-lag)
(lag tapers 10->4 to ride out late mask/v DMAs), one filler matmul
(vproj early, then qproj of later Sq chunks / oproj of finished chunks).
The cost model collapses the PE clock permanently after a single >~4us
idle gap, so the PE queue is kept stocked; gaps under ~3.3us are free.
The mask is DMA'd as 64 [128,512] column tiles so pass-0 mask arrival
(~0.63us/tile) outruns consumption (~1.04us/tile); k/q/v projection PSUM
tiles share the alpha pool's [P,1024] buffers to avoid ping-pong stalls.
"""

import numpy as np

B, S, D, H = 2, 2048, 1024, 16
DH = D // H  # 64
HEADS_PER_CORE = 4
N_CORES = 8
NQ = 4  # Sq chunks of 512
NSK = 16  # Sk chunks of 128
KC = 8  # D chunks of 128

_NC = None  # cached compiled bass program


def _build():
    import concourse.mybir as mybir
    import concourse.tile as tile
    from concourse import bacc

    F32 = mybir.dt.float32
    BF16 = mybir.dt.bfloat16
    P = 128

    nc = bacc.Bacc("TRN2")

    qT = nc.dram_tensor("qT", [D, S], BF16, kind="ExternalInput")
    kT = nc.dram_tensor("kT", [D, S], BF16, kind="ExternalInput")
    vT = nc.dram_tensor("vT", [D, S], BF16, kind="ExternalInput")
    maskT = nc.dram_tensor("maskT", [S, S], BF16, kind="ExternalInput")
    wqT = nc.dram_tensor("wqT", [D, 256], BF16, kind="ExternalInput")
    wkT = nc.dram_tensor("wkT", [D, 256], BF16, kind="ExternalInput")
    wvT = nc.dram_tensor("wvT", [D, 256], BF16, kind="ExternalInput")
    woT = nc.dram_tensor("woT", [256, D], BF16, kind="ExternalInput")
    wqb = nc.dram_tensor("wqb", [256], F32, kind="ExternalInput")
    wkb = nc.dram_tensor("wkb", [256], F32, kind="ExternalInput")
    wvb = nc.dram_tensor("wvb", [256], F32, kind="ExternalInput")
    ident = nc.dram_tensor("ident", [128, 128], BF16, kind="ExternalInput")
    out = nc.dram_tensor("out", [S, D], BF16, kind="ExternalOutput")

    AF = mybir.ActivationFunctionType
    MUL = mybir.AluOpType.mult
    ADD = mybir.AluOpType.add

    with tile.TileContext(nc) as tc:
        with (
            tc.tile_pool(name="persist", bufs=1) as persist,
            tc.tile_pool(name="kst", bufs=8) as kst,
            tc.tile_pool(name="vst", bufs=16) as vst,
            tc.tile_pool(name="qs", bufs=16) as qs,
            tc.tile_pool(name="mcp", bufs=32) as mcp,
            tc.tile_pool(name="pbuf", bufs=14) as pbuf,
            tc.tile_pool(name="obuf", bufs=2) as obuf,
            tc.tile_pool(name="nbuf", bufs=1) as nbuf,
            tc.tile_pool(name="ps_proj", bufs=2, space="PSUM") as ps_proj,
            tc.tile_pool(name="ps_alpha", bufs=2, space="PSUM") as ps_alpha,
            tc.tile_pool(name="ps_xp", bufs=2, space="PSUM") as ps_xp,
        ):
            # ---------------- DMA stream 1: k, q(nq0), v halves -------------
            ident_sb = persist.tile([P, 128], BF16, tag="ident")
            nc.sync.dma_start(ident_sb[:], ident[:])
            wkT_sb = persist.tile([P, KC, 256], BF16, tag="wkT")
            nc.sync.dma_start(wkT_sb[:], wkT[:].rearrange("(kc p) m -> p kc m", p=P))
            wkb_sb = persist.tile([P, 2], F32, tag="wkb")
            nc.sync.dma_start(wkb_sb[:], wkb[:].rearrange("(pr p) -> p pr", p=P))
            kch = []
            for kc in range(KC):
                t = kst.tile([P, S], BF16, tag="kch", name=f"kch{kc}")
                nc.sync.dma_start(t[:], kT[P * kc : P * (kc + 1), :])
                kch.append(t)

            wqT_sb = persist.tile([P, KC, 256], BF16, tag="wqT")
            nc.sync.dma_start(wqT_sb[:], wqT[:].rearrange("(kc p) m -> p kc m", p=P))
            wqb_sb = persist.tile([P, 2], F32, tag="wqb")
            nc.sync.dma_start(wqb_sb[:], wqb[:].rearrange("(pr p) -> p pr", p=P))
            qch0 = []
            for kc in range(KC):
                t = qs.tile([P, 512], BF16, tag="qs", name=f"q0_{kc}")
                nc.sync.dma_start(t[:], qT[P * kc : P * (kc + 1), 0:512])
                qch0.append(t)

            wvT_sb = persist.tile([P, KC, 256], BF16, tag="wvT")
            nc.sync.dma_start(wvT_sb[:], wvT[:].rearrange("(kc p) m -> p kc m", p=P))
            wvb_sb = persist.tile([P, 256], F32, tag="wvb")
            nc.sync.dma_start(wvb_sb[:], wvb[:][None, :].to_broadcast((P, 256)))

            # mask column tiles: mask_c[c][sk] covers maskT rows of Sk chunk
            # sk, Sq columns [512c, 512c+512)
            mask_c = [[None] * NSK for _ in range(NQ)]

            def dma_mask_col(c, lo=0, hi=NSK):
                for sk in range(lo, hi):
                    t = mcp.tile([P, 512], BF16, tag="mc", name=f"mc{c}_{sk}")
                    nc.sync.dma_start(
                        t[:], maskT[P * sk : P * (sk + 1), 512 * c : 512 * (c + 1)]
                    )
                    mask_c[c][sk] = t

            vch = [[None] * KC for _ in range(2)]

            def dma_v_half(half):
                for kc in range(KC):
                    t = vst.tile([P, 1024], BF16, tag="vch", name=f"vch{half}_{kc}")
                    nc.sync.dma_start(
                        t[:], vT[P * kc : P * (kc + 1), 1024 * half : 1024 * (half + 1)]
                    )
                    vch[half][kc] = t

            # interleave mask col 0 with the v halves: both gate the PVs
            dma_mask_col(0, 0, 8)
            dma_v_half(0)
            dma_mask_col(0, 8, NSK)
            dma_v_half(1)

            # ---------------- k projection (kps in the alpha pool) ----------
            # Two kc-outer passes (p0 then p1) so pass A overlaps the k DMA
            # arrivals; each pass holds two [P,1024] alpha-pool tiles, each
            # packing two nq chunks' 512-wide halves.
            kTp = [
                [
                    persist.tile([P, 512], BF16, tag=f"kTp{p}_{nq}", name=f"kTp{p}_{nq}")
                    for nq in range(NQ)
                ]
                for p in range(2)
            ]
            vp_sb = []
            for sk in range(NSK):
                vp = persist.tile([P, 4, 65], BF16, tag=f"vp{sk}", name=f"vp{sk}")
                nc.gpsimd.memset(vp[:], 1.0)  # ones column (col 64 per head)
                vp_sb.append(vp)
            # pass A (p0): two [P,1024] alpha-pool tiles, kc-outer so it is
            # paced by the k-chunk DMA arrivals. pass B (p1) interleaves one
            # kc behind pass A, filling the PE's DMA-pacing gaps; its PSUM
            # tiles live in ps_xp/ps_proj (both idle during projections) so
            # nothing ping-pongs on the alpha buffers.
            kpsA = [
                ps_alpha.tile([P, 1024], F32, tag="alpha", name=f"kpsA{pair}")
                for pair in range(2)
            ]
            def kproj_mm(p, nq, kc, dst):
                nc.tensor.matmul(
                    dst,
                    wkT_sb[:, kc, 128 * p : 128 * (p + 1)],
                    kch[kc][:, 512 * nq : 512 * (nq + 1)],
                    start=(kc == 0),
                    stop=(kc == KC - 1),
                )

            for kc in range(KC):
                for nq in range(NQ):
                    kproj_mm(
                        0, nq, kc, kpsA[nq // 2][:, 512 * (nq % 2) : 512 * (nq % 2 + 1)]
                    )
            for nq in range(NQ):
                nc.vector.tensor_scalar(
                    kTp[0][nq][:],
                    kpsA[nq // 2][:, 512 * (nq % 2) : 512 * (nq % 2 + 1)],
                    wkb_sb[:, 0:1],
                    None,
                    ADD,
                )
            # p1 halves: two sequential kc-sweeps through two ps_proj tiles
            for pair in range(2):
                kpsB = [
                    ps_proj.tile(
                        [P, 512], F32, tag="psproj", name=f"kpsB{2 * pair + j}"
                    )
                    for j in range(2)
                ]
                for kc in range(KC):
                    for j in range(2):
                        kproj_mm(1, 2 * pair + j, kc, kpsB[j][:])
                for j in range(2):
                    nc.vector.tensor_scalar(
                        kTp[1][2 * pair + j][:],
                        kpsB[j][:],
                        wkb_sb[:, 1:2],
                        None,
                        ADD,
                    )

            # ---------------- q projection ----------------
            qTp = [
                [
                    persist.tile([P, 512], BF16, tag=f"qTp{p}_{nq}", name=f"qTp{p}_{nq}")
                    for nq in range(NQ)
                ]
                for p in range(2)
            ]

            def qproj_half_units(nq, qtiles, col_off, pps, p):
                """Per-kc units projecting one p-half of q chunk nq."""

                def mk(kc):
                    def emit():
                        nc.tensor.matmul(
                            pps,
                            wqT_sb[:, kc, 128 * p : 128 * (p + 1)],
                            qtiles[kc][:, col_off : col_off + 512],
                            start=(kc == 0),
                            stop=(kc == KC - 1),
                        )
                        if kc == KC - 1:
                            nc.vector.tensor_scalar(
                                qTp[p][nq][:],
                                pps,
                                wqb_sb[:, p : p + 1],
                                None,
                                ADD,
                            )

                    return emit

                return [mk(kc) for kc in range(KC)]

            def qproj_units(nq, qtiles, col_off):
                pps = [
                    ps_proj.tile([P, 512], F32, tag="psproj", name=f"qps{nq}_{p}")[:]
                    for p in range(2)
                ]
                u0 = qproj_half_units(nq, qtiles, col_off, pps[0], 0)
                u1 = qproj_half_units(nq, qtiles, col_off, pps[1], 1)
                return [
                    (lambda a=a, b=b: (a(), b(), None)[2]) for a, b in zip(u0, u1)
                ]

            # p0 half inline (QK block (0,0) only needs qTp[0][0]); p1 half
            # becomes the first filler units inside the attention stream.
            qps0 = [
                ps_proj.tile([P, 512], F32, tag="psproj", name=f"qps0_{p}")[:]
                for p in range(2)
            ]
            for u in qproj_half_units(0, qch0, 0, qps0[0], 0):
                u()
            q0p1_units = qproj_half_units(0, qch0, 0, qps0[1], 1)

            # stream-2 DMAs are emitted inside the slot loop (sched entries)
            # so buffer-recycling dependencies match emission order.
            woT_sb = persist.tile([P, 2, D], BF16, tag="woT")
            qch1, qch2, qch3 = [], [], []

            def dma_q_chunk(dst, col0):
                for kc in range(KC):
                    t = qs.tile([P, 512], BF16, tag="qs", name=f"q{col0}_{kc}")
                    nc.sync.dma_start(
                        t[:], qT[P * kc : P * (kc + 1), col0 : col0 + 512]
                    )
                    dst.append(t)

            # ------------- flat-slot attention pipeline + fillers -----------
            xnorm = [
                [
                    persist.tile([P, 512], BF16, tag=f"xn{p}_{nq}", name=f"xn{p}_{nq}")
                    for nq in range(NQ)
                ]
                for p in range(2)
            ]

            def vproj_unit(sk):
                half, skl = sk // 8, sk % 8

                def emit():
                    vp_ps = ps_proj.tile([P, 512], F32, tag="psproj", name=f"vpps{sk}")
                    for kc in range(KC):
                        nc.tensor.matmul(
                            vp_ps[:, 0:256],
                            vch[half][kc][:, P * skl : P * (skl + 1)],
                            wvT_sb[:, kc],
                            start=(kc == 0),
                            stop=(kc == KC - 1),
                        )
                    nc.vector.tensor_tensor(
                        vp_sb[sk][:, :, 0:64],
                        vp_ps[:, 0:256].rearrange("p (h d) -> p h d", h=4),
                        wvb_sb[:].rearrange("p (h d) -> p h d", h=4),
                        ADD,
                    )

                return emit

            def oproj_units(nq):
                units = []
                for ml in range(4):
                    m = 4 * nq + ml
                    osb = obuf.tile([P, D], BF16, tag="osb", name=f"osb{m}")
                    for d in range(2):
                        ops = ps_proj.tile(
                            [P, 512], F32, tag="psproj", name=f"ops{m}_{d}"
                        )

                        def emit_mm(pr2, m=m, ml=ml, d=d, osb=osb, ops=ops):
                            nc.tensor.matmul(
                                ops[:],
                                xnorm[pr2][nq][:, P * ml : P * (ml + 1)],
                                woT_sb[:, pr2, 512 * d : 512 * (d + 1)],
                                start=(pr2 == 0),
                                stop=(pr2 == 1),
                            )
                            if pr2 == 1:
                                dst = osb[:, 512 * d : 512 * (d + 1)]
                                if nq == NQ - 1:
                                    nc.scalar.copy(out=dst, in_=ops[:])
                                else:
                                    nc.vector.tensor_copy(out=dst, in_=ops[:])
                                if d == 1:
                                    nc.sync.dma_start(
                                        out[P * m : P * (m + 1), :], osb[:]
                                    )

                        units.append(lambda f=emit_mm: f(0))
                        units.append(lambda f=emit_mm: f(1))
                return units

            # filler schedule: (slot, lazy factory). DMA batches are also
            # sched units: emitting them inside the slot stream keeps the
            # buffer-recycle dependencies consistent with their readers.
            def dma_unit(fn):
                return lambda: [fn]

            sched = []
            sched.append((0, lambda: q0p1_units))
            sched.append((8, dma_unit(lambda: dma_q_chunk(qch1, 512))))
            for sk in range(NSK):
                sched.append((10 + sk, lambda sk=sk: [vproj_unit(sk)]))
            sched.append((10, dma_unit(lambda: dma_mask_col(1))))
            sched.append(
                (
                    11,
                    dma_unit(
                        lambda: (
                            nc.sync.dma_start(
                                woT_sb[:],
                                woT[:].rearrange("(pr p) m -> p pr m", p=P),
                            ),
                            dma_q_chunk(qch2, 1024),
                        )
                    ),
                )
            )
            sched.append((26, lambda: qproj_units(1, qch1, 0)))
            sched.append((32, dma_unit(lambda: dma_q_chunk(qch3, 1536))))
            sched.append((33, dma_unit(lambda: dma_mask_col(2))))
            sched.append((43, lambda: qproj_units(2, qch2, 0)))
            sched.append((52, lambda: oproj_units(0)))
            sched.append((64, dma_unit(lambda: dma_mask_col(3))))
            sched.append((69, lambda: qproj_units(3, qch3, 0)))
            sched.append((78, lambda: oproj_units(1)))
            sched.append((101, lambda: oproj_units(2)))
            sched.sort(key=lambda x: x[0])

            # PV slot map: big lag early (rides out late mask/v DMA), base 4
            # in steady state, +3 cushion at block starts so the previous
            # block's normalize chain can release the xps buffers in time.
            pv_slot = []
            for i in range(128):
                base = i + (12 if i <= 15 else 4)
                prev = pv_slot[-1] if pv_slot else 0
                if i % 16 == 0 and i > 0:
                    prev += 3
                pv_slot.append(max(base, prev))

            # slot -> list of PV indices
            pv_at = {}
            for i in range(128):
                pv_at.setdefault(pv_slot[i], []).append(i)

            psb_store = {}
            xps_store = {}
            pending = []
            si = 0

            def normalize(b):
                """x arrives [sq, dh+den] packed in two [P,512] PSUM tiles:
                per-partition divide on DVE, then PE transposes rebuild the
                [2h*64dh, sq] xnorm layout via one borrowed ps_proj tile."""
                nq, pr = b // 2, b % 2
                xbt = xps_store.pop(b)
                xnb = [[None] * 2 for _ in range(4)]
                for sb in range(4):
                    for h in range(2):
                        col = 256 * (sb % 2) + 128 * h
                        r = nbuf.tile(
                            [P, 1], F32, tag="rr", bufs=8, name=f"r{b}_{sb}_{h}"
                        )
                        nc.vector.reciprocal(
                            r[:], xbt[sb // 2][:, col + 64 : col + 65]
                        )
                        t = nbuf.tile(
                            [P, 64], BF16, tag="xnb", bufs=8, name=f"xnb{b}_{sb}_{h}"
                        )
                        nc.vector.tensor_scalar(
                            t[:], xbt[sb // 2][:, col : col + 64], r[:], None, MUL
                        )
                        xnb[sb][h] = t
                trt = ps_proj.tile([P, 512], F32, tag="psproj", name=f"trt{b}")
                for sb in range(4):
                    for h in range(2):
                        nc.tensor.matmul(
                            trt[64 * h : 64 * h + 64, 128 * sb : 128 * (sb + 1)],
                            xnb[sb][h][:],
                            ident_sb[:],
                            start=True,
                            stop=True,
                        )
                nc.vector.tensor_copy(out=xnorm[pr][nq][:], in_=trt[:])

            NSLOT = pv_slot[-1] + 1
            for s in range(NSLOT):
                if s < 128:
                    b, sk = s // 16, s % 16
                    nq, pr = b // 2, b % 2
                    alpha = ps_alpha.tile(
                        [P, 1024], F32, tag="alpha", name=f"al{b}_{sk}"
                    )
                    for h in range(2):
                        nc.tensor.matmul(
                            alpha[:, 512 * h : 512 * (h + 1)],
                            kTp[pr][sk // 4][
                                64 * h : 64 * h + 64,
                                P * (sk % 4) : P * (sk % 4 + 1),
                            ],
                            qTp[pr][nq][64 * h : 64 * h + 64, :],
                            start=True,
                            stop=True,
                            tile_position=(64 * h, 0),
                        )
                    psb = pbuf.tile([P, 1024], BF16, tag="psb", name=f"psb{b}_{sk}")
                    nc.scalar.activation(psb[:], alpha[:], AF.Exp)
                    nc.vector.tensor_tensor(
                        psb[:].rearrange("p (h n) -> p h n", h=2),
                        psb[:].rearrange("p (h n) -> p h n", h=2),
                        mask_c[nq][sk][:][:, None, :].to_broadcast((P, 2, 512)),
                        MUL,
                    )
                    psb_store[s] = psb
                for i in pv_at.get(s, []):
                    b, sk = i // 16, i % 16
                    nq, pr = b // 2, b % 2
                    if sk == 0:
                        xps_store[b] = [
                            ps_xp.tile([P, 512], F32, tag="xps", name=f"xb{b}_{j}")
                            for j in range(2)
                        ]
                    psb = psb_store.pop(i)
                    # the four groups in each [P,512] tile share one PSUM
                    # bank: only the first write carries start=True (it
                    # zeroes the whole bank); hw wipes sibling partial sums
                    # if several groups in a bank assert start.
                    for sb in range(4):
                        for h in range(2):
                            col = 256 * (sb % 2) + 128 * h
                            nc.tensor.matmul(
                                xps_store[b][sb // 2][:, col : col + 65],
                                psb[:, 512 * h + 128 * sb : 512 * h + 128 * (sb + 1)],
                                vp_sb[sk][:, 2 * pr + h],
                                start=(sk == 0 and h == 0 and sb % 2 == 0),
                                stop=(sk == NSK - 1),
                                skip_group_check=True,
                            )
                    if sk == NSK - 1:
                        normalize(b)
                while si < len(sched) and sched[si][0] <= s:
                    pending.extend(sched[si][1]())
                    si += 1
                n_emit = min(2, len(pending))
                for _ in range(n_emit):
                    pending.pop(0)()
            while si < len(sched):
                pending.extend(sched[si][1]())
                si += 1
            for u in pending:
                u()
            for u in oproj_units(3):
                u()

    nc.finalize()
    return nc


def _get_nc():
    global _NC
    if _NC is None:
        _NC = _build()
    return _NC


def _prep_inputs(q, k, v, mask, wq_w, wq_b, wk_w, wk_b, wv_w, wv_b, wo_w, wo_b):
    import ml_dtypes

    f32 = np.float32
    bf16 = ml_dtypes.bfloat16
    q = np.asarray(q, f32)
    k = np.asarray(k, f32)
    v = np.asarray(v, f32)
    mask = np.asarray(mask)
    wq_w = np.asarray(wq_w, f32)
    wk_w = np.asarray(wk_w, f32)
    wv_w = np.asarray(wv_w, f32)
    wo_w = np.asarray(wo_w, f32)

    qTb = [np.ascontiguousarray(q[b].T).astype(bf16) for b in range(B)]
    kTb = [np.ascontiguousarray(k[b].T).astype(bf16) for b in range(B)]
    vTb = [np.ascontiguousarray(v[b].T).astype(bf16) for b in range(B)]
    maskTb = [
        np.ascontiguousarray((~mask[b, 0]).T).astype(bf16) for b in range(B)
    ]

    ident = np.eye(128, dtype=bf16)
    in_maps = []
    for c in range(N_CORES):
        b = c // 4
        g = c % 4
        rows = slice(256 * g, 256 * (g + 1))
        in_maps.append(
            {
                "qT": qTb[b],
                "kT": kTb[b],
                "vT": vTb[b],
                "maskT": maskTb[b],
                "wqT": np.ascontiguousarray(wq_w[rows, :].T).astype(bf16),
                "wkT": np.ascontiguousarray(wk_w[rows, :].T).astype(bf16),
                "wvT": np.ascontiguousarray(wv_w[rows, :].T).astype(bf16),
                "woT": np.ascontiguousarray(wo_w[:, rows].T).astype(bf16),
                "wqb": np.ascontiguousarray(np.asarray(wq_b, f32)[rows]),
                "wkb": np.ascontiguousarray(np.asarray(wk_b, f32)[rows]),
                "wvb": np.ascontiguousarray(np.asarray(wv_b, f32)[rows]),
                "ident": ident,
            }
        )
    return in_maps


def run(inputs, trace=False):
    """Run the kernel; returns (output, BassKernelResults)."""
    from concourse.bass_utils import run_bass_kernel_spmd

    in_maps = _prep_inputs(**inputs)
    nc = _get_nc()
    res = None
    last_exc = None
    for attempt in range(3):
        try:
            res = run_bass_kernel_spmd(
                nc, in_maps, core_ids=list(range(N_CORES)), trace=trace
            )
            break
        except Exception as e:  # transient device/tunnel failures
            last_exc = e
            try:
                import jax

                jax.clear_caches()
                try:
                    jax.extend.backend.clear_backends()
                except Exception:
                    from jax._src import api as _jax_api

                    _jax_api.clear_backends()
            except Exception:
                pass
            import time as _time

            _time.sleep(2.0 * (attempt + 1))
    if res is None:
        raise last_exc
    wo_b = np.asarray(inputs["wo_b"], np.float32)
    out = np.zeros((B, S, D), np.float32)
    for b in range(B):
        acc = np.zeros((S, D), np.float32)
        for g in range(4):
            acc += np.asarray(res.results[4 * b + g]["out"], np.float32)
        out[b] = acc + wo_b[None, :]
    return out, res


def kernel(**inputs) -> np.ndarray:
    out, _ = run(inputs, trace=False)
    return out
